# revision 16
# baseline (speedup 1.0000x reference)
"""Trainium2 Bass kernel for nn_AttnDecoder (protein conv encoder + GO attention).

Strategy: data-parallel over batch — 32 samples -> 4 per NeuronCore x 8 cores,
all parameters replicated. The GO embedding gather (32 rows of a 30000-row
table) and the tiny aa-embedding gather (26x5 table) are done host-side as
part of input distribution.

Math restructure (per sample):
  - conv1 as a dense K=80 matmul: host ships the im2col of the embedded
    sequence (rows (k,e) = x_emb[e, t+k], 75 used + 5 pad) in bf16; weights
    w1e[(k,e), o] = conv1_w[o,e,k].  Single-pass K=80 matmuls (no
    accumulation chain) cover all 15 taps at once.
  - conv2 = 60 PSUM-accumulating bf16 matmuls per output tile (4 in-channel
    chunks x 15 taps), rhs read directly from x1 with shifted column windows.
  - x2 is evacuated from PSUM as bf16 (relu+bias on the DVE/Act engines);
    the energy matmuls then run at 1 cycle/row instead of fp32's 4.
  - attention: v = go @ attn_w (device matmul, fp32, once per core);
    energies_n = sum_q v_q enc[q,n] as 16 M=1 bf16 matmuls accumulating into
    one PSUM bank (enc is a row-major view of x2: enc[(c,s), n] =
    x2[c, 2 + 252*s + n]).  attn_b shifts all energies of a sample equally,
    which softmax ignores, so it is dropped.
  - softmax over 252 sections on-device; context = DVE multiply + row reduce.
"""

import numpy as np
import ml_dtypes

import concourse.bass as bass
import concourse.mybir as mybir
import concourse.tile as tile
from concourse.bass_utils import run_bass_kernel_spmd

# ---- problem constants (must match the reference) ----
B, L = 32, 2048
NCORES = 8
BPC = B // NCORES          # samples per core
AA_VOCAB, AA_EMB = 26, 5
C = 256                    # conv2 out channels
C2 = 512                   # conv1 out channels
S = 8                      # section size
CS = C * S                 # 2048
GO = 256                   # go embedding dim
KS = 15                    # conv kernel size
L1 = L - KS + 1            # 2034, conv1 output length
P2 = L1 - KS + 1           # 2020, conv2 output length
NSEC = P2 // S             # 252 sections
HEAD = (P2 % S) // 2       # 2, head trim of the section view
KE = 80                    # conv1 im2col rows: 15 taps x 5 emb = 75 -> pad 80
KC1 = 4                    # conv2 in-channel chunks: 512 -> 4x128
N1_TILES = (0, 512, 1024, 1536)
N2_TILES = (0, 505, 1010, 1515)

F32 = mybir.dt.float32
BF16 = mybir.dt.bfloat16
TRACE = False
LAST_RESULT = None

_NC_CACHE = {}


def _build():
    nc = bass.Bass()
    im2_d = nc.dram_tensor("im2", [BPC, KE, L1], BF16, kind="ExternalInput")
    w1e_d = nc.dram_tensor("w1e", [KE, C2], BF16, kind="ExternalInput")
    b1_d = nc.dram_tensor("b1", [C2], F32, kind="ExternalInput")
    w2t_d = nc.dram_tensor("w2t", [KC1, 128, KS, C], BF16, kind="ExternalInput")
    b2_d = nc.dram_tensor("b2", [C], F32, kind="ExternalInput")
    goT_d = nc.dram_tensor("goT", [GO, BPC], F32, kind="ExternalInput")
    attnw_d = nc.dram_tensor("attnw", [GO, CS], F32, kind="ExternalInput")
    out_d = nc.dram_tensor("out", [BPC, CS], F32, kind="ExternalOutput")

    RELU = mybir.ActivationFunctionType.Relu
    EXP = mybir.ActivationFunctionType.Exp
    AX = mybir.AxisListType.X

    with (
        tile.TileContext(nc) as tc,
        tc.tile_pool(name="singles", bufs=1) as singles,
        tc.tile_pool(name="persamp", bufs=2) as persamp,
        tc.tile_pool(name="big", bufs=2) as big,
        tc.tile_pool(name="mm", bufs=4, space="PSUM") as mmpool,
        tc.tile_pool(name="spsum", bufs=2, space="PSUM") as spsum,
        tc.tile_pool(name="dram", bufs=2, space="DRAM") as dpool,
    ):
        # ---- resident weights / constants ----
        w1sb = singles.tile([KE, C2], BF16)
        nc.sync.dma_start(w1sb, w1e_d[:, :])
        # kc-major layout: the first conv2 chain (kc=0) only waits on the
        # first of these four 975KB DMAs.
        w2sb = singles.tile([128, KC1, KS, C], BF16)
        for kc in range(KC1):
            nc.sync.dma_start(w2sb[:, kc], w2t_d[kc])
        b1sb = singles.tile([128, 4], F32)
        nc.sync.dma_start(b1sb, b1_d.rearrange("(c p) -> p c", p=128))
        b2sb = singles.tile([128, 2], F32)
        nc.sync.dma_start(b2sb, b2_d.rearrange("(c p) -> p c", p=128))
        gosb = singles.tile([128, 2, BPC], F32)
        nc.sync.dma_start(gosb, goT_d.rearrange("(c p) b -> p c b", p=128))
        awsb = singles.tile([128, 2, CS], F32)
        nc.sync.dma_start(awsb, attnw_d.rearrange("(c p) n -> p c n", p=128))
        onesb = singles.tile([1, 128], BF16)
        nc.vector.memset(onesb, 1.0)

        # V = go_sel @ attn_w is emitted inside sample 0, after its conv2,
        # so the PE is not stalled at startup behind the attnw DMA; the DRAM
        # roundtrip gives the per-sample [128, 2, 8] channel-major lhsT.
        vdram = dpool.tile([BPC, CS], BF16)

        def emit_v():
            vsb = singles.tile([BPC, CS], BF16)
            for n in range(4):
                vps = mmpool.tile([128, 512], F32, tag="mm512")
                for c in range(2):
                    nc.tensor.matmul(
                        vps[:BPC, :],
                        gosb[:, c, :],
                        awsb[:, c, 512 * n : 512 * (n + 1)],
                        start=(c == 0),
                        stop=(c == 1),
                    )
                nc.scalar.copy(vsb[:, 512 * n : 512 * (n + 1)], vps[:BPC, :])
            nc.sync.dma_start(vdram[:, :], vsb[:])

        prev = None
        for b in range(BPC):
            x1 = _sample_conv1(
                nc, b, im2_d, persamp, big, mmpool, w1sb, b1sb,
            )
            if prev is not None:
                _attn_tail(nc, *prev, out_d, persamp, onesb, spsum)
            prev = _sample_conv2_energies(
                nc, b, x1, persamp, big, mmpool, spsum,
                w2sb, b2sb, vdram, emit_v if b == 0 else None,
            )
        _attn_tail(nc, *prev, out_d, persamp, onesb, spsum)
    return nc


def _sample_conv1(nc, b, im2_d, persamp, big, mmpool, w1sb, b1sb):
    RELU = mybir.ActivationFunctionType.Relu

    # host-precomputed im2col of the embedded sequence: one contiguous DMA
    rhs1 = big.tile([KE, L1], BF16, tag="rhs1")
    nc.sync.dma_start(rhs1, im2_d[b])

    # conv1 + bias + relu -> x1 [512ch, 2034] (K=80 single pass)
    x1 = big.tile([128, 4, L1], BF16, tag="x1")
    for n0 in N1_TILES:
        for m in range(4):
            nn = min(512, L1 - n0)
            ps = mmpool.tile([128, 512], F32, tag="mm512")
            nc.tensor.matmul(
                ps[:, :nn],
                w1sb[:, 128 * m : 128 * (m + 1)],
                rhs1[:, n0 : n0 + nn],
                start=True,
                stop=True,
            )
            nc.scalar.activation(
                out=x1[:, m, n0 : n0 + nn],
                in_=ps[:, :nn],
                func=RELU,
                bias=b1sb[:, m : m + 1],
                scale=1.0,
            )
    return x1


def _sample_conv2_energies(nc, b, x1, persamp, big, mmpool, spsum,
                           w2sb, b2sb, vdram, emit_v=None):
    RELU = mybir.ActivationFunctionType.Relu

    def fetch_vmat():
        vmat = persamp.tile([128, 2, S], BF16, tag="vmat")
        with nc.allow_non_contiguous_dma(reason="per-channel gather of V"):
            nc.sync.dma_start(
                vmat, vdram[b].rearrange("(c p s) -> p c s", p=128, s=S)
            )
        return vmat

    # sample 0 must wait for V to exist; later samples prefetch up front
    vmat = fetch_vmat() if emit_v is None else None

    # conv2 + bias + relu -> x2 [256ch, 2020] bf16
    x2 = big.tile([128, 2, P2], BF16, tag="x2")
    for m in range(2):
        for n0 in N2_TILES:
            nn = 505
            ps2 = mmpool.tile([128, 512], F32, tag="mm512")
            idx = 0
            for kc in range(KC1):
                for k in range(KS):
                    nc.tensor.matmul(
                        ps2[:, :nn],
                        w2sb[:, kc, k, 128 * m : 128 * (m + 1)],
                        x1[:, kc, n0 + k : n0 + k + nn],
                        start=(idx == 0),
                        stop=(idx == KC1 * KS - 1),
                    )
                    idx += 1
            nc.scalar.activation(
                out=x2[:, m, n0 : n0 + nn],
                in_=ps2[:, :nn],
                func=RELU,
                bias=b2sb[:, m : m + 1],
                scale=1.0,
            )

    if emit_v is not None:
        emit_v()
        vmat = fetch_vmat()

    # energies[n] = sum_q v_q * enc[q, n] (bf16, 1 cyc/row)
    eng = spsum.tile([1, NSEC], F32, tag="eng")
    idx = 0
    for c in range(2):
        for s in range(S):
            nc.tensor.matmul(
                eng,
                vmat[:, c, s : s + 1],
                x2[:, c, HEAD + NSEC * s : HEAD + NSEC * s + NSEC],
                start=(idx == 0),
                stop=(idx == 2 * S - 1),
            )
            idx += 1
    return b, eng, x2


def _attn_tail(nc, b, eng, x2, out_d, persamp, onesb, spsum):
    """Softmax + attn broadcast + context for sample b.  Emitted after the
    NEXT sample's conv1 so the ones-matmul broadcast (which waits on the
    softmax chain) does not stall the PE queue."""
    EXP = mybir.ActivationFunctionType.Exp
    AX = mybir.AxisListType.X

    negmax = persamp.tile([1, 1], F32)
    nc.vector.reduce_max(negmax, eng, axis=AX, negate=True)
    expd = persamp.tile([1, NSEC], F32)
    nc.scalar.activation(out=expd, in_=eng, func=EXP, bias=negmax, scale=1.0)
    ssum = persamp.tile([1, 1], F32)
    nc.vector.reduce_sum(ssum, expd, axis=AX)
    rsum = persamp.tile([1, 1], F32)
    nc.vector.reciprocal(rsum, ssum)
    attn_t = persamp.tile([1, NSEC], BF16)
    nc.vector.tensor_scalar_mul(attn_t, expd, rsum)

    # broadcast attn over 128 partitions with a K=1 ones matmul
    # (single-partition DMAs fail NEFF loading in this environment)
    aps = spsum.tile([128, NSEC], F32, tag="abc")
    nc.tensor.matmul(aps, onesb, attn_t, start=True, stop=True)
    attnb = persamp.tile([128, NSEC], BF16)
    nc.scalar.copy(attnb, aps)

    # ctx[(c,s)] = sum_n attn[n] * enc[(c,s), n]
    ctx_t = persamp.tile([128, 2, S], F32)
    for c in range(2):
        for s in range(S):
            tmp = persamp.tile([128, NSEC], F32, tag="tmp")
            nc.vector.tensor_mul(
                tmp,
                x2[:, c, HEAD + NSEC * s : HEAD + NSEC * s + NSEC],
                attnb,
            )
            nc.vector.reduce_sum(ctx_t[:, c, s : s + 1], tmp, axis=AX)
    with nc.allow_non_contiguous_dma(reason="sectioned layout store"):
        nc.sync.dma_start(
            out_d[b].rearrange("(c p s) -> p c s", p=128, s=S), ctx_t
        )


def _hoist_excess_waits(nc, cap=1):
    """Walrus codegen fits only one sem-wait slot on a Matmult (the LDWEIGHTS
    struct), but Tile attaches one wait per producer processor.  Hoist the
    excess waits onto standalone EventSemaphore instructions inserted just
    before the offender on the same engine queue — queues execute in order,
    so this is semantically identical."""
    import json as _json

    bir = _json.loads(nc.to_json_bytes())
    ctr = [0]

    def fix_block(b):
        insts = b.get("instructions")
        if insts:
            new = []
            for ins in insts:
                si = ins.get("sync_info")
                waits = (si or {}).get("on_wait") or []
                if len(waits) > cap:
                    keep = waits[len(waits) - cap :] if cap else []
                    for w in waits[: len(waits) - cap]:
                        ctr[0] += 1
                        new.append(
                            {
                                "debug": ins.get("debug"),
                                "engine": ins["engine"],
                                "ins": [],
                                "name": f"hoistw-{ctr[0]}",
                                "opcode": "EventSemaphore",
                                "outs": [],
                                "sync_info": {"on_update": [], "on_wait": [w]},
                            }
                        )
                    si["on_wait"] = keep
                new.append(ins)
            b["instructions"] = new
        for sb in b.get("blocks") or []:
            fix_block(sb)

    for fnc in bir["functions"]:
        for b in fnc["blocks"]:
            fix_block(b)
    patched = _json.dumps(bir).encode()
    nc.to_json_bytes = lambda: patched
    return ctr[0]


def get_nc():
    if "v2" not in _NC_CACHE:
        nc = _build()
        n = _hoist_excess_waits(nc)
        print(f"hoisted {n} excess matmul waits", flush=True)
        _NC_CACHE["v2"] = nc
    return _NC_CACHE["v2"]


def prep_in_maps(
    input_seq,
    input_go_term,
    aa_emb,
    conv1_w,
    conv1_b,
    conv2_w,
    conv2_b,
    go_table,
    attn_w,
    attn_b,
):
    seq = np.asarray(input_seq).astype(np.int64)
    got = np.asarray(input_go_term).astype(np.int64)
    aa = np.asarray(aa_emb).astype(np.float32)
    w1 = np.asarray(conv1_w).astype(np.float32)
    b1 = np.asarray(conv1_b).astype(np.float32)
    w2 = np.asarray(conv2_w).astype(np.float32)
    b2 = np.asarray(conv2_b).astype(np.float32)
    gt = np.asarray(go_table).astype(np.float32)
    aw = np.asarray(attn_w).astype(np.float32)
    # attn_b shifts all of a sample's energies by one constant -> softmax
    # invariant, so it never reaches the device.

    bf = ml_dtypes.bfloat16

    # conv1 weights as [(k,e), o] with vocab folded via the host-side gather
    w1e = np.zeros((KE, C2), np.float32)
    w1e[: KS * AA_EMB] = w1.transpose(2, 1, 0).reshape(KS * AA_EMB, C2)
    w1e = w1e.astype(bf)

    # [KC1, 128, KS, C]: w2t[kc, p, k, o] = w2[o, 128*kc+p, k]
    w2t = np.ascontiguousarray(
        w2.transpose(1, 2, 0).reshape(KC1, 128, KS, C)
    ).astype(bf)

    # host im2col of the embedded sequence: [B, 80, L1] bf16
    xe = aa.astype(bf).astype(np.float32)[seq]        # [B, L, 5] quantized like w
    xe = np.ascontiguousarray(xe.transpose(0, 2, 1))  # [B, 5, L]
    win = np.lib.stride_tricks.sliding_window_view(xe, L1, axis=2)  # [B,5,15,L1]
    im2 = np.zeros((B, KE, L1), np.float32)
    im2[:, : KS * AA_EMB] = win.transpose(0, 2, 1, 3).reshape(B, KS * AA_EMB, L1)
    im2 = im2.astype(bf)

    go_sel = gt[got]  # [B, 256]

    in_maps = []
    for core in range(NCORES):
        sl = slice(core * BPC, (core + 1) * BPC)
        in_maps.append(
            {
                "im2": np.ascontiguousarray(im2[sl]),
                "w1e": w1e,
                "b1": b1,
                "w2t": w2t,
                "b2": b2,
                "goT": np.ascontiguousarray(go_sel[sl].T.astype(np.float32)),
                "attnw": aw,
            }
        )
    return in_maps


def kernel(**inputs):
    global LAST_RESULT
    nc = get_nc()
    in_maps = prep_in_maps(**inputs)
    res = run_bass_kernel_spmd(
        nc, in_maps, core_ids=list(range(NCORES)), trace=TRACE
    )
    LAST_RESULT = res
    return np.concatenate([r["out"] for r in res.results], axis=0)


# revision 21
# speedup vs baseline: 1.0468x; 1.0468x over previous
"""Trainium2 Bass kernel for nn_AttnDecoder (protein conv encoder + GO attention).

Strategy: data-parallel over batch — 32 samples -> 4 per NeuronCore x 8 cores,
all parameters replicated. The GO embedding gather (32 rows of a 30000-row
table) and the tiny aa-embedding gather (26x5 table) are done host-side as
part of input distribution.

Math restructure (per sample):
  - conv1 as a dense K=80 matmul: host ships the im2col of the embedded
    sequence (rows (k,e) = x_emb[e, t+k], 75 used + 5 pad) in bf16; weights
    w1e[(k,e), o] = conv1_w[o,e,k].  Single-pass K=80 matmuls (no
    accumulation chain) cover all 15 taps at once.
  - conv2 = 60 PSUM-accumulating bf16 matmuls per output tile (4 in-channel
    chunks x 15 taps), rhs read directly from x1 with shifted column windows.
  - x2 is evacuated from PSUM as bf16 (relu+bias on the DVE/Act engines);
    the energy matmuls then run at 1 cycle/row instead of fp32's 4.
  - attention: v = go @ attn_w (device matmul, fp32, once per core);
    energies_n = sum_q v_q enc[q,n] as 16 M=1 bf16 matmuls accumulating into
    one PSUM bank (enc is a row-major view of x2: enc[(c,s), n] =
    x2[c, 2 + 252*s + n]).  attn_b shifts all energies of a sample equally,
    which softmax ignores, so it is dropped.
  - softmax over 252 sections on-device; context = DVE multiply + row reduce.
"""

import numpy as np
import ml_dtypes

import concourse.bass as bass
import concourse.mybir as mybir
import concourse.tile as tile
from concourse.bass_utils import run_bass_kernel_spmd

# ---- problem constants (must match the reference) ----
B, L = 32, 2048
NCORES = 8
BPC = B // NCORES          # samples per core
AA_VOCAB, AA_EMB = 26, 5
C = 256                    # conv2 out channels
C2 = 512                   # conv1 out channels
S = 8                      # section size
CS = C * S                 # 2048
GO = 256                   # go embedding dim
KS = 15                    # conv kernel size
L1 = L - KS + 1            # 2034, conv1 output length
P2 = L1 - KS + 1           # 2020, conv2 output length
NSEC = P2 // S             # 252 sections
HEAD = (P2 % S) // 2       # 2, head trim of the section view
KE = 80                    # conv1 im2col rows: 15 taps x 5 emb = 75 -> pad 80
KC1 = 4                    # conv2 in-channel chunks: 512 -> 4x128
N1_TILES = (0, 512, 1024, 1536)
N2_TILES = (0, 505, 1010, 1515)

F32 = mybir.dt.float32
BF16 = mybir.dt.bfloat16
TRACE = False
LAST_RESULT = None

_NC_CACHE = {}


def _build():
    nc = bass.Bass()
    im2_d = nc.dram_tensor("im2", [BPC, KE, L1], BF16, kind="ExternalInput")
    w1e_d = nc.dram_tensor("w1e", [KE, C2], BF16, kind="ExternalInput")
    b1_d = nc.dram_tensor("b1", [C2], F32, kind="ExternalInput")
    w2t_d = nc.dram_tensor("w2t", [KC1, 128, KS, C], BF16, kind="ExternalInput")
    b2_d = nc.dram_tensor("b2", [C], F32, kind="ExternalInput")
    goT_d = nc.dram_tensor("goT", [GO, BPC], F32, kind="ExternalInput")
    attnw_d = nc.dram_tensor("attnw", [GO, CS], F32, kind="ExternalInput")
    out_d = nc.dram_tensor("out", [BPC, 128, 2 * S], F32, kind="ExternalOutput")

    RELU = mybir.ActivationFunctionType.Relu
    EXP = mybir.ActivationFunctionType.Exp
    AX = mybir.AxisListType.X

    with (
        tile.TileContext(nc) as tc,
        tc.tile_pool(name="singles", bufs=1) as singles,
        tc.tile_pool(name="persamp", bufs=2) as persamp,
        tc.tile_pool(name="big", bufs=2) as big,
        tc.tile_pool(name="mm", bufs=4, space="PSUM") as mmpool,
        tc.tile_pool(name="spsum", bufs=2, space="PSUM") as spsum,
        tc.tile_pool(name="dram", bufs=2, space="DRAM") as dpool,
    ):
        # ---- resident weights / constants ----
        w1sb = singles.tile([KE, C2], BF16)
        nc.sync.dma_start(w1sb, w1e_d[:, :])
        # kc-major layout: the first conv2 chain (kc=0) only waits on the
        # first of these four 975KB DMAs.
        w2sb = singles.tile([128, KC1, KS, C], BF16)
        for kc in range(KC1):
            nc.sync.dma_start(w2sb[:, kc], w2t_d[kc])
        b1sb = singles.tile([128, 4], F32)
        nc.sync.dma_start(b1sb, b1_d.rearrange("(c p) -> p c", p=128))
        b2sb = singles.tile([128, 2], F32)
        nc.sync.dma_start(b2sb, b2_d.rearrange("(c p) -> p c", p=128))
        onesb = singles.tile([1, 128], BF16)
        nc.vector.memset(onesb, 1.0)

        # V = go_sel @ attn_w is emitted inside sample 0, after its conv2,
        # so the PE is not stalled at startup behind the attnw DMA (which is
        # also only started there, keeping the early DMA bandwidth for the
        # conv weights).  vdram holds V in the per-sample [128, (c s)]
        # channel-major layout so the per-sample fetch is contiguous.
        vdram = dpool.tile([BPC, 128, 2 * S], BF16)

        def emit_v():
            gosb = singles.tile([128, 2, BPC], F32)
            nc.sync.dma_start(gosb, goT_d.rearrange("(c p) b -> p c b", p=128))
            awsb = singles.tile([128, 2, CS], F32)
            nc.sync.dma_start(awsb, attnw_d.rearrange("(c p) n -> p c n", p=128))
            # vsb free layout [c, p, s] == flat V index q = c*1024 + p*8 + s
            vsb = singles.tile([BPC, 2, 128, S], BF16)
            for n in range(4):
                vps = mmpool.tile([128, 512], F32, tag="mm512")
                for c in range(2):
                    nc.tensor.matmul(
                        vps[:BPC, :],
                        gosb[:, c, :],
                        awsb[:, c, 512 * n : 512 * (n + 1)],
                        start=(c == 0),
                        stop=(c == 1),
                    )
                nc.scalar.copy(
                    vsb[:, n // 2, 64 * (n % 2) : 64 * (n % 2) + 64, :],
                    vps[:BPC, :],
                )
            with nc.allow_non_contiguous_dma(reason="permute V to channel-major"):
                for c in range(2):
                    nc.sync.dma_start(
                        vdram[:, :, S * c : S * (c + 1)], vsb[:, c]
                    )

        prev = None
        for b in range(BPC):
            x1 = _sample_conv1(
                nc, b, im2_d, persamp, big, mmpool, w1sb, b1sb,
            )
            if prev is not None:
                _attn_tail(nc, *prev, out_d, persamp, onesb, spsum)
            prev = _sample_conv2_energies(
                nc, b, x1, persamp, big, mmpool, spsum,
                w2sb, b2sb, vdram, emit_v if b == 0 else None,
            )
        _attn_tail(nc, *prev, out_d, persamp, onesb, spsum)
    return nc


def _sample_conv1(nc, b, im2_d, persamp, big, mmpool, w1sb, b1sb):
    RELU = mybir.ActivationFunctionType.Relu

    # host-precomputed im2col of the embedded sequence: one contiguous DMA
    rhs1 = big.tile([KE, L1], BF16, tag="rhs1")
    nc.sync.dma_start(rhs1, im2_d[b])

    # conv1 + bias + relu -> x1 [512ch, 2034] (K=80 single pass)
    x1 = big.tile([128, 4, L1], BF16, tag="x1")
    for n0 in N1_TILES:
        for m in range(4):
            nn = min(512, L1 - n0)
            ps = mmpool.tile([128, 512], F32, tag="mm512")
            nc.tensor.matmul(
                ps[:, :nn],
                w1sb[:, 128 * m : 128 * (m + 1)],
                rhs1[:, n0 : n0 + nn],
                start=True,
                stop=True,
            )
            if m % 2 == 0:
                nc.scalar.activation(
                    out=x1[:, m, n0 : n0 + nn],
                    in_=ps[:, :nn],
                    func=RELU,
                    bias=b1sb[:, m : m + 1],
                    scale=1.0,
                )
            else:
                nc.vector.tensor_scalar(
                    out=x1[:, m, n0 : n0 + nn],
                    in0=ps[:, :nn],
                    scalar1=b1sb[:, m : m + 1],
                    scalar2=0.0,
                    op0=mybir.AluOpType.add,
                    op1=mybir.AluOpType.max,
                )
    return x1


def _sample_conv2_energies(nc, b, x1, persamp, big, mmpool, spsum,
                           w2sb, b2sb, vdram, emit_v=None):
    RELU = mybir.ActivationFunctionType.Relu

    def fetch_vmat():
        vmat = persamp.tile([128, 2, S], BF16, tag="vmat")
        nc.sync.dma_start(vmat, vdram[b])
        return vmat

    # sample 0 must wait for V to exist; later samples prefetch up front
    vmat = fetch_vmat() if emit_v is None else None

    # conv2 + bias + relu -> x2 [256ch, 2020] bf16
    x2 = big.tile([128, 2, P2], BF16, tag="x2")
    for m in range(2):
        for n0 in N2_TILES:
            nn = 505
            ps2 = mmpool.tile([128, 512], F32, tag="mm512")
            idx = 0
            for kc in range(KC1):
                for k in range(KS):
                    nc.tensor.matmul(
                        ps2[:, :nn],
                        w2sb[:, kc, k, 128 * m : 128 * (m + 1)],
                        x1[:, kc, n0 + k : n0 + k + nn],
                        start=(idx == 0),
                        stop=(idx == KC1 * KS - 1),
                    )
                    idx += 1
            nc.scalar.activation(
                out=x2[:, m, n0 : n0 + nn],
                in_=ps2[:, :nn],
                func=RELU,
                bias=b2sb[:, m : m + 1],
                scale=1.0,
            )

    if emit_v is not None:
        emit_v()
        vmat = fetch_vmat()

    # energies[n] = sum_q v_q * enc[q, n] (bf16, 1 cyc/row)
    eng = spsum.tile([1, NSEC], F32, tag="eng")
    idx = 0
    for c in range(2):
        for s in range(S):
            nc.tensor.matmul(
                eng,
                vmat[:, c, s : s + 1],
                x2[:, c, HEAD + NSEC * s : HEAD + NSEC * s + NSEC],
                start=(idx == 0),
                stop=(idx == 2 * S - 1),
            )
            idx += 1
    return b, eng, x2


def _attn_tail(nc, b, eng, x2, out_d, persamp, onesb, spsum):
    """Softmax + attn broadcast + context for sample b.  Emitted after the
    NEXT sample's conv1 so the ones-matmul broadcast (which waits on the
    softmax chain) does not stall the PE queue."""
    EXP = mybir.ActivationFunctionType.Exp
    AX = mybir.AxisListType.X

    negmax = persamp.tile([1, 1], F32)
    nc.vector.reduce_max(negmax, eng, axis=AX, negate=True)
    expd = persamp.tile([1, NSEC], F32)
    nc.scalar.activation(out=expd, in_=eng, func=EXP, bias=negmax, scale=1.0)
    ssum = persamp.tile([1, 1], F32)
    nc.vector.reduce_sum(ssum, expd, axis=AX)
    rsum = persamp.tile([1, 1], F32)
    nc.vector.reciprocal(rsum, ssum)
    attn_t = persamp.tile([1, NSEC], BF16)
    nc.vector.tensor_scalar_mul(attn_t, expd, rsum)

    # broadcast attn over 128 partitions with a K=1 ones matmul
    # (single-partition DMAs fail NEFF loading in this environment)
    aps = spsum.tile([128, NSEC], F32, tag="abc")
    nc.tensor.matmul(aps, onesb, attn_t, start=True, stop=True)
    attnb = persamp.tile([128, NSEC], BF16)
    nc.scalar.copy(attnb, aps)

    # ctx[(c,s)] = sum_n attn[n] * enc[(c,s), n]
    ctx_t = persamp.tile([128, 2, S], F32)
    for c in range(2):
        for s in range(S):
            tmp = persamp.tile([128, NSEC], BF16, tag="tmp")
            nc.vector.tensor_mul(
                tmp,
                x2[:, c, HEAD + NSEC * s : HEAD + NSEC * s + NSEC],
                attnb,
            )
            nc.vector.reduce_sum(ctx_t[:, c, s : s + 1], tmp, axis=AX)
    nc.sync.dma_start(out_d[b], ctx_t)


def _hoist_excess_waits(nc, cap=1):
    """Walrus codegen fits only one sem-wait slot on a Matmult (the LDWEIGHTS
    struct), but Tile attaches one wait per producer processor.  Hoist the
    excess waits onto standalone EventSemaphore instructions inserted just
    before the offender on the same engine queue — queues execute in order,
    so this is semantically identical."""
    import json as _json

    bir = _json.loads(nc.to_json_bytes())
    ctr = [0]

    def fix_block(b):
        insts = b.get("instructions")
        if insts:
            new = []
            for ins in insts:
                si = ins.get("sync_info")
                waits = (si or {}).get("on_wait") or []
                if len(waits) > cap:
                    keep = waits[len(waits) - cap :] if cap else []
                    for w in waits[: len(waits) - cap]:
                        ctr[0] += 1
                        new.append(
                            {
                                "debug": ins.get("debug"),
                                "engine": ins["engine"],
                                "ins": [],
                                "name": f"hoistw-{ctr[0]}",
                                "opcode": "EventSemaphore",
                                "outs": [],
                                "sync_info": {"on_update": [], "on_wait": [w]},
                            }
                        )
                    si["on_wait"] = keep
                new.append(ins)
            b["instructions"] = new
        for sb in b.get("blocks") or []:
            fix_block(sb)

    for fnc in bir["functions"]:
        for b in fnc["blocks"]:
            fix_block(b)
    patched = _json.dumps(bir).encode()
    nc.to_json_bytes = lambda: patched
    return ctr[0]


def get_nc():
    if "v2" not in _NC_CACHE:
        nc = _build()
        n = _hoist_excess_waits(nc)
        print(f"hoisted {n} excess matmul waits", flush=True)
        _NC_CACHE["v2"] = nc
    return _NC_CACHE["v2"]


def prep_in_maps(
    input_seq,
    input_go_term,
    aa_emb,
    conv1_w,
    conv1_b,
    conv2_w,
    conv2_b,
    go_table,
    attn_w,
    attn_b,
):
    seq = np.asarray(input_seq).astype(np.int64)
    got = np.asarray(input_go_term).astype(np.int64)
    aa = np.asarray(aa_emb).astype(np.float32)
    w1 = np.asarray(conv1_w).astype(np.float32)
    b1 = np.asarray(conv1_b).astype(np.float32)
    w2 = np.asarray(conv2_w).astype(np.float32)
    b2 = np.asarray(conv2_b).astype(np.float32)
    gt = np.asarray(go_table).astype(np.float32)
    aw = np.asarray(attn_w).astype(np.float32)
    # attn_b shifts all of a sample's energies by one constant -> softmax
    # invariant, so it never reaches the device.

    bf = ml_dtypes.bfloat16

    # conv1 weights as [(k,e), o] with vocab folded via the host-side gather
    w1e = np.zeros((KE, C2), np.float32)
    w1e[: KS * AA_EMB] = w1.transpose(2, 1, 0).reshape(KS * AA_EMB, C2)
    w1e = w1e.astype(bf)

    # [KC1, 128, KS, C]: w2t[kc, p, k, o] = w2[o, 128*kc+p, k]
    w2t = np.ascontiguousarray(
        w2.transpose(1, 2, 0).reshape(KC1, 128, KS, C)
    ).astype(bf)

    # host im2col of the embedded sequence: [B, 80, L1] bf16
    xe = aa.astype(bf).astype(np.float32)[seq]        # [B, L, 5] quantized like w
    xe = np.ascontiguousarray(xe.transpose(0, 2, 1))  # [B, 5, L]
    win = np.lib.stride_tricks.sliding_window_view(xe, L1, axis=2)  # [B,5,15,L1]
    im2 = np.zeros((B, KE, L1), np.float32)
    im2[:, : KS * AA_EMB] = win.transpose(0, 2, 1, 3).reshape(B, KS * AA_EMB, L1)
    im2 = im2.astype(bf)

    go_sel = gt[got]  # [B, 256]

    in_maps = []
    for core in range(NCORES):
        sl = slice(core * BPC, (core + 1) * BPC)
        in_maps.append(
            {
                "im2": np.ascontiguousarray(im2[sl]),
                "w1e": w1e,
                "b1": b1,
                "w2t": w2t,
                "b2": b2,
                "goT": np.ascontiguousarray(go_sel[sl].T.astype(np.float32)),
                "attnw": aw,
            }
        )
    return in_maps


def kernel(**inputs):
    global LAST_RESULT
    nc = get_nc()
    in_maps = prep_in_maps(**inputs)
    res = run_bass_kernel_spmd(
        nc, in_maps, core_ids=list(range(NCORES)), trace=TRACE
    )
    LAST_RESULT = res
    dev = np.concatenate([r["out"] for r in res.results], axis=0)
    # dev[b, p, c*S+s] -> out[b, (c*128+p)*S+s]
    dev = dev.reshape(B, 128, 2, S).transpose(0, 2, 1, 3).reshape(B, CS)
    return np.ascontiguousarray(dev)


# revision 24
# speedup vs baseline: 1.0627x; 1.0152x over previous
"""Trainium2 Bass kernel for nn_AttnDecoder (protein conv encoder + GO attention).

Strategy: data-parallel over batch — 32 samples -> 4 per NeuronCore x 8 cores,
all parameters replicated. The GO embedding gather (32 rows of a 30000-row
table) and the tiny aa-embedding gather (26x5 table) are done host-side as
part of input distribution.

Math restructure (per sample):
  - conv1 as a dense K=80 matmul: host ships the im2col of the embedded
    sequence (rows (k,e) = x_emb[e, t+k], 75 used + 5 pad) in bf16; weights
    w1e[(k,e), o] = conv1_w[o,e,k].  Single-pass K=80 matmuls (no
    accumulation chain) cover all 15 taps at once.
  - conv2 = 60 PSUM-accumulating bf16 matmuls per output tile (4 in-channel
    chunks x 15 taps), rhs read directly from x1 with shifted column windows.
  - x2 is evacuated from PSUM as bf16 (relu+bias on the DVE/Act engines);
    the energy matmuls then run at 1 cycle/row instead of fp32's 4.
  - attention: v = go @ attn_w (device matmul, fp32, once per core);
    energies_n = sum_q v_q enc[q,n] as 16 M=1 bf16 matmuls accumulating into
    one PSUM bank (enc is a row-major view of x2: enc[(c,s), n] =
    x2[c, 2 + 252*s + n]).  attn_b shifts all energies of a sample equally,
    which softmax ignores, so it is dropped.
  - softmax over 252 sections on-device; context = DVE multiply + row reduce.
"""

import numpy as np
import ml_dtypes

import concourse.bass as bass
import concourse.mybir as mybir
import concourse.tile as tile
from concourse.bass_utils import run_bass_kernel_spmd

# ---- problem constants (must match the reference) ----
B, L = 32, 2048
NCORES = 8
BPC = B // NCORES          # samples per core
AA_VOCAB, AA_EMB = 26, 5
C = 256                    # conv2 out channels
C2 = 512                   # conv1 out channels
S = 8                      # section size
CS = C * S                 # 2048
GO = 256                   # go embedding dim
KS = 15                    # conv kernel size
L1 = L - KS + 1            # 2034, conv1 output length
P2 = L1 - KS + 1           # 2020, conv2 output length
NSEC = P2 // S             # 252 sections
HEAD = (P2 % S) // 2       # 2, head trim of the section view
KE = 80                    # conv1 im2col rows: 15 taps x 5 emb = 75 -> pad 80
KC1 = 4                    # conv2 in-channel chunks: 512 -> 4x128
N1_TILES = (0, 512, 1024, 1536)
N2_TILES = (0, 505, 1010, 1515)

F32 = mybir.dt.float32
BF16 = mybir.dt.bfloat16
TRACE = False
LAST_RESULT = None

_NC_CACHE = {}


def _build():
    nc = bass.Bass()
    im2_d = nc.dram_tensor("im2", [BPC, KE, L1], BF16, kind="ExternalInput")
    w1e_d = nc.dram_tensor("w1e", [KE, C2], BF16, kind="ExternalInput")
    b1_d = nc.dram_tensor("b1", [C2], F32, kind="ExternalInput")
    w2t_d = nc.dram_tensor("w2t", [KC1, 128, KS, C], BF16, kind="ExternalInput")
    b2_d = nc.dram_tensor("b2", [C], F32, kind="ExternalInput")
    goT_d = nc.dram_tensor("goT", [GO, BPC], F32, kind="ExternalInput")
    attnw_d = nc.dram_tensor("attnw", [GO, CS], F32, kind="ExternalInput")
    out_d = nc.dram_tensor("out", [BPC, 128, 2 * S], F32, kind="ExternalOutput")

    RELU = mybir.ActivationFunctionType.Relu
    EXP = mybir.ActivationFunctionType.Exp
    AX = mybir.AxisListType.X

    with (
        tile.TileContext(nc) as tc,
        tc.tile_pool(name="singles", bufs=1) as singles,
        tc.tile_pool(name="persamp", bufs=2) as persamp,
        tc.tile_pool(name="big", bufs=2) as big,
        tc.tile_pool(name="mm", bufs=4, space="PSUM") as mmpool,
        tc.tile_pool(name="spsum", bufs=2, space="PSUM") as spsum,
        tc.tile_pool(name="dram", bufs=2, space="DRAM") as dpool,
    ):
        # ---- resident weights / constants ----
        # DMA queue plan (two HWDGE queues: SP via nc.sync, Act via
        # nc.scalar).  conv2's first chain needs w2[kc0][taps 0:8] first, so
        # that chunk leads the otherwise-idle Act queue; the small tiles the
        # first conv1 needs (w1, im2col of sample 0, biases) lead SP.
        w2sb = singles.tile([128, KC1, KS, C], BF16)
        nc.scalar.dma_start(w2sb[:, 0, :8], w2t_d[0, :, :8])
        w1sb = singles.tile([KE, C2], BF16)
        nc.sync.dma_start(w1sb, w1e_d[:, :])
        rhs1_s0 = big.tile([KE, L1], BF16, tag="rhs1")
        nc.sync.dma_start(rhs1_s0, im2_d[0])
        b1sb = singles.tile([128, 4], F32)
        nc.sync.dma_start(b1sb, b1_d.rearrange("(c p) -> p c", p=128))
        b2sb = singles.tile([128, 2], F32)
        nc.sync.dma_start(b2sb, b2_d.rearrange("(c p) -> p c", p=128))
        for kc in range(1, KC1):
            nc.sync.dma_start(w2sb[:, kc, :8], w2t_d[kc, :, :8])
        for kc in range(KC1):
            nc.scalar.dma_start(w2sb[:, kc, 8:], w2t_d[kc, :, 8:])
        onesb = singles.tile([1, 128], BF16)
        nc.vector.memset(onesb, 1.0)

        # V = go_sel @ attn_w is emitted inside sample 0, after its conv2,
        # so the PE is not stalled at startup behind the attnw DMA (which is
        # also only started there, keeping the early DMA bandwidth for the
        # conv weights).  vdram holds V in the per-sample [128, (c s)]
        # channel-major layout so the per-sample fetch is contiguous.
        vdram = dpool.tile([BPC, 128, 2 * S], BF16)

        def emit_v():
            gosb = singles.tile([128, 2, BPC], F32)
            nc.sync.dma_start(gosb, goT_d.rearrange("(c p) b -> p c b", p=128))
            awsb = singles.tile([128, 2, CS], F32)
            nc.sync.dma_start(awsb, attnw_d.rearrange("(c p) n -> p c n", p=128))
            # vsb free layout [c, p, s] == flat V index q = c*1024 + p*8 + s
            vsb = singles.tile([BPC, 2, 128, S], BF16)
            for n in range(4):
                vps = mmpool.tile([128, 512], F32, tag="mm512")
                for c in range(2):
                    nc.tensor.matmul(
                        vps[:BPC, :],
                        gosb[:, c, :],
                        awsb[:, c, 512 * n : 512 * (n + 1)],
                        start=(c == 0),
                        stop=(c == 1),
                    )
                nc.scalar.copy(
                    vsb[:, n // 2, 64 * (n % 2) : 64 * (n % 2) + 64, :],
                    vps[:BPC, :],
                )
            with nc.allow_non_contiguous_dma(reason="permute V to channel-major"):
                for c in range(2):
                    nc.sync.dma_start(
                        vdram[:, :, S * c : S * (c + 1)], vsb[:, c]
                    )

        prev = None
        for b in range(BPC):
            x1 = _sample_conv1(
                nc, b, im2_d, persamp, big, mmpool, w1sb, b1sb,
                rhs1_s0 if b == 0 else None,
            )
            if prev is not None:
                _attn_tail(nc, *prev, out_d, persamp, onesb, spsum)
            prev = _sample_conv2_energies(
                nc, b, x1, persamp, big, mmpool, spsum,
                w2sb, b2sb, vdram, emit_v if b == 0 else None,
            )
        _attn_tail(nc, *prev, out_d, persamp, onesb, spsum)
    return nc


def _sample_conv1(nc, b, im2_d, persamp, big, mmpool, w1sb, b1sb, rhs1=None):
    RELU = mybir.ActivationFunctionType.Relu

    if rhs1 is None:
        # host-precomputed im2col of the embedded sequence: one contiguous DMA
        rhs1 = big.tile([KE, L1], BF16, tag="rhs1")
        nc.sync.dma_start(rhs1, im2_d[b])

    # conv1 + bias + relu -> x1 [512ch, 2034] (K=80 single pass)
    x1 = big.tile([128, 4, L1], BF16, tag="x1")
    for n0 in N1_TILES:
        for m in range(4):
            nn = min(512, L1 - n0)
            ps = mmpool.tile([128, 512], F32, tag="mm512")
            nc.tensor.matmul(
                ps[:, :nn],
                w1sb[:, 128 * m : 128 * (m + 1)],
                rhs1[:, n0 : n0 + nn],
                start=True,
                stop=True,
            )
            if m % 2 == 0:
                nc.scalar.activation(
                    out=x1[:, m, n0 : n0 + nn],
                    in_=ps[:, :nn],
                    func=RELU,
                    bias=b1sb[:, m : m + 1],
                    scale=1.0,
                )
            else:
                nc.vector.tensor_scalar(
                    out=x1[:, m, n0 : n0 + nn],
                    in0=ps[:, :nn],
                    scalar1=b1sb[:, m : m + 1],
                    scalar2=0.0,
                    op0=mybir.AluOpType.add,
                    op1=mybir.AluOpType.max,
                )
    return x1


def _sample_conv2_energies(nc, b, x1, persamp, big, mmpool, spsum,
                           w2sb, b2sb, vdram, emit_v=None):
    RELU = mybir.ActivationFunctionType.Relu

    def fetch_vmat():
        vmat = persamp.tile([128, 2, S], BF16, tag="vmat")
        nc.sync.dma_start(vmat, vdram[b])
        return vmat

    # sample 0 must wait for V to exist; later samples prefetch up front
    vmat = fetch_vmat() if emit_v is None else None

    # conv2 + bias + relu -> x2 [256ch, 2020] bf16
    x2 = big.tile([128, 2, P2], BF16, tag="x2")
    for m in range(2):
        for n0 in N2_TILES:
            nn = 505
            ps2 = mmpool.tile([128, 512], F32, tag="mm512")
            idx = 0
            for kc in range(KC1):
                for k in range(KS):
                    nc.tensor.matmul(
                        ps2[:, :nn],
                        w2sb[:, kc, k, 128 * m : 128 * (m + 1)],
                        x1[:, kc, n0 + k : n0 + k + nn],
                        start=(idx == 0),
                        stop=(idx == KC1 * KS - 1),
                    )
                    idx += 1
            nc.scalar.activation(
                out=x2[:, m, n0 : n0 + nn],
                in_=ps2[:, :nn],
                func=RELU,
                bias=b2sb[:, m : m + 1],
                scale=1.0,
            )

    if emit_v is not None:
        emit_v()
        vmat = fetch_vmat()

    # energies[n] = sum_q v_q * enc[q, n] (bf16, 1 cyc/row)
    eng = spsum.tile([1, NSEC], F32, tag="eng")
    idx = 0
    for c in range(2):
        for s in range(S):
            nc.tensor.matmul(
                eng,
                vmat[:, c, s : s + 1],
                x2[:, c, HEAD + NSEC * s : HEAD + NSEC * s + NSEC],
                start=(idx == 0),
                stop=(idx == 2 * S - 1),
            )
            idx += 1
    return b, eng, x2


def _attn_tail(nc, b, eng, x2, out_d, persamp, onesb, spsum):
    """Softmax + attn broadcast + context for sample b.  Emitted after the
    NEXT sample's conv1 so the ones-matmul broadcast (which waits on the
    softmax chain) does not stall the PE queue."""
    EXP = mybir.ActivationFunctionType.Exp
    AX = mybir.AxisListType.X

    negmax = persamp.tile([1, 1], F32)
    nc.vector.reduce_max(negmax, eng, axis=AX, negate=True)
    expd = persamp.tile([1, NSEC], F32)
    nc.scalar.activation(out=expd, in_=eng, func=EXP, bias=negmax, scale=1.0)
    ssum = persamp.tile([1, 1], F32)
    nc.vector.reduce_sum(ssum, expd, axis=AX)
    rsum = persamp.tile([1, 1], F32)
    nc.vector.reciprocal(rsum, ssum)
    attn_t = persamp.tile([1, NSEC], BF16)
    nc.vector.tensor_scalar_mul(attn_t, expd, rsum)

    # broadcast attn over 128 partitions with a K=1 ones matmul
    # (single-partition DMAs fail NEFF loading in this environment)
    aps = spsum.tile([128, NSEC], F32, tag="abc")
    nc.tensor.matmul(aps, onesb, attn_t, start=True, stop=True)
    attnb = persamp.tile([128, NSEC], BF16)
    nc.scalar.copy(attnb, aps)

    # ctx[(c,s)] = sum_n attn[n] * enc[(c,s), n]
    ctx_t = persamp.tile([128, 2, S], F32)
    for c in range(2):
        for s in range(S):
            tmp = persamp.tile([128, NSEC], BF16, tag="tmp")
            nc.vector.tensor_mul(
                tmp,
                x2[:, c, HEAD + NSEC * s : HEAD + NSEC * s + NSEC],
                attnb,
            )
            nc.vector.reduce_sum(ctx_t[:, c, s : s + 1], tmp, axis=AX)
    nc.sync.dma_start(out_d[b], ctx_t)


def _hoist_excess_waits(nc, cap=1):
    """Walrus codegen fits only one sem-wait slot on a Matmult (the LDWEIGHTS
    struct), but Tile attaches one wait per producer processor.  Hoist the
    excess waits onto standalone EventSemaphore instructions inserted just
    before the offender on the same engine queue — queues execute in order,
    so this is semantically identical."""
    import json as _json

    bir = _json.loads(nc.to_json_bytes())
    ctr = [0]

    def fix_block(b):
        insts = b.get("instructions")
        if insts:
            new = []
            for ins in insts:
                si = ins.get("sync_info")
                waits = (si or {}).get("on_wait") or []
                if len(waits) > cap:
                    keep = waits[len(waits) - cap :] if cap else []
                    for w in waits[: len(waits) - cap]:
                        ctr[0] += 1
                        new.append(
                            {
                                "debug": ins.get("debug"),
                                "engine": ins["engine"],
                                "ins": [],
                                "name": f"hoistw-{ctr[0]}",
                                "opcode": "EventSemaphore",
                                "outs": [],
                                "sync_info": {"on_update": [], "on_wait": [w]},
                            }
                        )
                    si["on_wait"] = keep
                new.append(ins)
            b["instructions"] = new
        for sb in b.get("blocks") or []:
            fix_block(sb)

    for fnc in bir["functions"]:
        for b in fnc["blocks"]:
            fix_block(b)
    patched = _json.dumps(bir).encode()
    nc.to_json_bytes = lambda: patched
    return ctr[0]


def get_nc():
    if "v2" not in _NC_CACHE:
        nc = _build()
        n = _hoist_excess_waits(nc)
        print(f"hoisted {n} excess matmul waits", flush=True)
        _NC_CACHE["v2"] = nc
    return _NC_CACHE["v2"]


def prep_in_maps(
    input_seq,
    input_go_term,
    aa_emb,
    conv1_w,
    conv1_b,
    conv2_w,
    conv2_b,
    go_table,
    attn_w,
    attn_b,
):
    seq = np.asarray(input_seq).astype(np.int64)
    got = np.asarray(input_go_term).astype(np.int64)
    aa = np.asarray(aa_emb).astype(np.float32)
    w1 = np.asarray(conv1_w).astype(np.float32)
    b1 = np.asarray(conv1_b).astype(np.float32)
    w2 = np.asarray(conv2_w).astype(np.float32)
    b2 = np.asarray(conv2_b).astype(np.float32)
    gt = np.asarray(go_table).astype(np.float32)
    aw = np.asarray(attn_w).astype(np.float32)
    # attn_b shifts all of a sample's energies by one constant -> softmax
    # invariant, so it never reaches the device.

    bf = ml_dtypes.bfloat16

    # conv1 weights as [(k,e), o] with vocab folded via the host-side gather
    w1e = np.zeros((KE, C2), np.float32)
    w1e[: KS * AA_EMB] = w1.transpose(2, 1, 0).reshape(KS * AA_EMB, C2)
    w1e = w1e.astype(bf)

    # [KC1, 128, KS, C]: w2t[kc, p, k, o] = w2[o, 128*kc+p, k]
    w2t = np.ascontiguousarray(
        w2.transpose(1, 2, 0).reshape(KC1, 128, KS, C)
    ).astype(bf)

    # host im2col of the embedded sequence: [B, 80, L1] bf16
    xe = aa.astype(bf).astype(np.float32)[seq]        # [B, L, 5] quantized like w
    xe = np.ascontiguousarray(xe.transpose(0, 2, 1))  # [B, 5, L]
    win = np.lib.stride_tricks.sliding_window_view(xe, L1, axis=2)  # [B,5,15,L1]
    im2 = np.zeros((B, KE, L1), np.float32)
    im2[:, : KS * AA_EMB] = win.transpose(0, 2, 1, 3).reshape(B, KS * AA_EMB, L1)
    im2 = im2.astype(bf)

    go_sel = gt[got]  # [B, 256]

    in_maps = []
    for core in range(NCORES):
        sl = slice(core * BPC, (core + 1) * BPC)
        in_maps.append(
            {
                "im2": np.ascontiguousarray(im2[sl]),
                "w1e": w1e,
                "b1": b1,
                "w2t": w2t,
                "b2": b2,
                "goT": np.ascontiguousarray(go_sel[sl].T.astype(np.float32)),
                "attnw": aw,
            }
        )
    return in_maps


def kernel(**inputs):
    global LAST_RESULT
    nc = get_nc()
    in_maps = prep_in_maps(**inputs)
    res = run_bass_kernel_spmd(
        nc, in_maps, core_ids=list(range(NCORES)), trace=TRACE
    )
    LAST_RESULT = res
    dev = np.concatenate([r["out"] for r in res.results], axis=0)
    # dev[b, p, c*S+s] -> out[b, (c*128+p)*S+s]
    dev = dev.reshape(B, 128, 2, S).transpose(0, 2, 1, 3).reshape(B, CS)
    return np.ascontiguousarray(dev)
